# revision 47
# baseline (speedup 1.0000x reference)
"""Trainium2 Bass kernel for nn_CapRNNModelHelper (bi-GRU + capsule routing).

Sharding: data-parallel over batch across 8 cores (16 batch rows per core).
Everything else (embedding table, GRU weights, capsule weights) replicated.

Per-core pipeline (fp16 matmul operands, f32 accumulation):
  1. indirect-DMA gather of f16 embedding rows (token order s-major)
  2. PE-transpose (f16 psum) -> e.T  [300, ntok] fp16
  3. x_proj matmuls (fp16) -> xp_rz (fp16) + xp_n (f32), biases folded,
     z blocks negated so sigmoid gives w = 1-z directly
  4. 256-step fused bidirectional GRU scan: per step the xp_rz slice is
     PSUM-accumulated via an identity matmul, gates matmul on top (fp16
     weights, fp16 h mirror), sigmoid/tanh on ACT, update on DVE+GpSimd
  5. capsule matmul (fp16) -> u_hat [sb, (k,i)160] f16
  6. 5-iter dynamic routing (selector matmuls for sequence reductions,
     iter-0 coupling fold, exp/ln-based 1/sqrt to stay in one ACT table)
  7. final linear -> out [16, 2]
"""

import numpy as np
from contextlib import ExitStack

import concourse.bass as bass
import concourse.tile as tile
from concourse import mybir
from concourse.bass import IndirectOffsetOnAxis
from concourse.bass_utils import run_bass_kernel_spmd
from concourse.tile_rust import add_dep_helper

F32 = mybir.dt.float32
BF16 = mybir.dt.float16
I32 = mybir.dt.int32
AF = mybir.ActivationFunctionType
OP = mybir.AluOpType
AX = mybir.AxisListType

VOCAB, D_W, H, S, B = 50000, 300, 128, 256, 128
NUM_CAP, DIM_CAP, ROUTINGS, EPS = 10, 16, 5, 1e-7
NCORES = 8
BL = B // NCORES          # 16 batch rows per core
NTOK = S * BL             # 4096 tokens per core
NGRP = NTOK // 128        # 32 gather groups of 128 tokens
NCH = NTOK // 512         # 8 x_proj chunks of 512 tokens
KCH = [(0, 128), (128, 128), (256, 44)]   # D_W split
G3 = 3 * H                # 384
GG = 1                    # token groups per indirect gather

RZW = 4 * BL              # 64   per-step rz width [rf zf rb zb]
NW = 2 * BL               # 32   per-step n width [nf nb]
PCH = 16                  # parallel chunks per direction in the scan
CCH = S // PCH            # 16 steps per chunk
WU = 9                    # warmup steps per chunk (GRU forgetting rebuilds h)
EXT = S + 2 * WU          # padded xp timeline
PB = PCH * BL             # 256  scan op width per direction
FSL = S + WU + 1          # forward h slots (position p at slot p+WU+1)
BSL = S + WU + 2          # backward h slots (position p at slot p+1)
# block index for (dir d, gate g): rz blocks 0..3, n blocks 0..1
_BLKRZ = {(0, 0): 0, (0, 1): 1, (1, 0): 2, (1, 1): 3}

# routing: groups handled by DVE vs GpSimd on the big elementwise ops
POOL_GRP = 10             # groups on GpSimd in big-op splits
DVE_GRP = NGRP - POOL_GRP


def _sub(base, off, dims):
    """Manual AP: base is a [128, X] AP; append free dims after partition."""
    return bass.AP(tensor=base.tensor, offset=base.offset + off,
                   ap=[base.ap[0]] + dims)


def _v(t, dims, off=0):
    return bass.AP(tensor=t.tensor, offset=t.offset + off,
                   ap=[t.ap[0]] + dims)


def _split_waits(nc, cap=1):
    """Hoist excess sync waits onto standalone event-semaphore ops.

    The walrus build on this stack accepts only `cap` sync-wait commands
    per ISA instruction; Tile can attach several. Event-semaphore ops on
    the same engine execute in queue order, so hoisting preserves
    semantics.
    """
    n = 0
    for fn in nc.m.functions:
        for bb in fn.blocks:
            out = []
            for ins in bb.instructions:
                si = ins.sync_info
                if si is not None and len(si.on_wait) > cap:
                    waits = list(si.on_wait)
                    keep = waits[len(waits) - cap:] if cap else []
                    for w in waits[:len(waits) - cap] if cap else waits:
                        n += 1
                        out.append(mybir.InstEventSemaphore(
                            name=f"wsplit-{n}", engine=ins.engine,
                            ins=[], outs=[],
                            sync_info=mybir.SyncInfo(on_wait=[w],
                                                     on_update=[])))
                    ins.sync_info = mybir.SyncInfo(
                        on_wait=keep, on_update=list(si.on_update))
                out.append(ins)
            bb.instructions = out
    return n


def _build(zero_bhn: bool, zero_bx: bool, debug: bool = False):
    nc = bass.Bass()
    if debug:
        dbg_hs_d = nc.declare_dram_parameter(
            "dbg_hs", [128, (FSL + BSL) * BL], BF16, True)

    xidx_d = nc.declare_dram_parameter("xidx", [128, NGRP], I32, False)
    emb_d = nc.declare_dram_parameter("emb", [VOCAB, D_W], BF16, False)
    wih_d = nc.declare_dram_parameter("wih", [2, D_W, G3], BF16, False)
    whh_d = nc.declare_dram_parameter("whh", [2, H, G3], BF16, False)
    biasx_d = nc.declare_dram_parameter("biasx", [128, 6], F32, False)
    bhn_d = nc.declare_dram_parameter("bhn", [128, 2], F32, False)
    wcap_d = nc.declare_dram_parameter("wcap", [2, H, 160], BF16, False)
    wlin_d = nc.declare_dram_parameter("wlin", [160, 2], F32, False)
    blin_d = nc.declare_dram_parameter("blin", [2, 1], F32, False)
    selB_d = nc.declare_dram_parameter("selB", [128, BL], F32, False)
    selT_d = nc.declare_dram_parameter("selT", [BL, 128], F32, False)
    ident_d = nc.declare_dram_parameter("ident", [128, 128], F32, False)
    out_d = nc.declare_dram_parameter("out", [BL, 2], F32, True)

    with tile.TileContext(nc) as tc, ExitStack() as ctx:
        const = ctx.enter_context(tc.tile_pool(name="const", bufs=1))
        bigxp = ctx.enter_context(tc.tile_pool(name="bigxp", bufs=1))
        bighs = ctx.enter_context(tc.tile_pool(name="bighs", bufs=1))
        work = ctx.enter_context(tc.tile_pool(name="work", bufs=3))

        # ---- constants to SBUF ----
        xidx = const.tile([128, NGRP], I32)
        # scalar HWDGE ring: empty at start, so the gathers unblock sooner
        nc.scalar.dma_start(out=xidx[:], in_=xidx_d[:, :])
        whh = const.tile([128, 2, G3], BF16)
        for d in range(2):
            nc.sync.dma_start(out=whh[:, d, :], in_=whh_d[d, :, :])
        biasx = const.tile([128, 6], F32)
        nc.sync.dma_start(out=biasx[:], in_=biasx_d[:, :])
        bhn = const.tile([128, 2], F32)
        nc.sync.dma_start(out=bhn[:], in_=bhn_d[:, :])
        wcap = const.tile([128, 2, 160], BF16)
        for k in range(2):
            nc.sync.dma_start(out=wcap[:, k, :], in_=wcap_d[k, :, :])
        wlin = const.tile([128, 2, 2], F32)        # chunk0 [:128], chunk1 [:32]
        nc.sync.dma_start(out=wlin[:, 0, :], in_=wlin_d[0:128, :])
        nc.sync.dma_start(out=wlin[:32, 1, :], in_=wlin_d[128:160, :])
        blin = const.tile([2, 1], F32)
        nc.sync.dma_start(out=blin[:], in_=blin_d[:, :])
        selBf = const.tile([128, BL], F32)
        nc.sync.dma_start(out=selBf[:], in_=selB_d[:, :])
        selB = const.tile([128, BL], BF16)
        nc.scalar.copy(selB[:], selBf[:])
        selT = const.tile([BL, 128], F32)
        nc.sync.dma_start(out=selT[:], in_=selT_d[:, :])
        selTb = const.tile([BL, 128], BF16)
        nc.scalar.copy(selTb[:], selT[:])
        ident = const.tile([128, 128], F32)
        nc.sync.dma_start(out=ident[:], in_=ident_d[:, :])
        identb = const.tile([128, 128], BF16)
        nc.scalar.copy(identb[:], ident[:])
        epst = const.tile([128, 1], F32)
        nc.vector.memset(epst[:], EPS)

        xprz = bigxp.tile([128, EXT * RZW], BF16)   # 35 KB/part
        xpn = bigxp.tile([128, EXT * NW], BF16)     # 17.5 KB/part
        HB0 = FSL * BL                              # backward region base
        hbf = bighs.tile([128, (FSL + BSL) * BL], BF16)  # 17.2 KB/part
        # warmup pads force h -> 0 exactly: r=sigmoid(-30)=0, w=sigmoid(30)=1,
        # xn=0  =>  h' = 1*tanh(0) + 0*h = 0
        for p0 in (0, S + WU):
            for blk, val in ((0, -30.0), (1, 30.0), (2, -30.0), (3, 30.0)):
                nc.vector.memset(_sub(xprz[:], p0 * RZW + blk * BL,
                                      [[RZW, WU], [1, BL]]), val)
            nc.gpsimd.memset(_sub(xpn[:], p0 * NW, [[1, WU * NW]]), 0.0)

        # ---- phases B+C: gather + transpose + x_proj (single pass) ----
        with tc.tile_pool(name="bc", bufs=1) as bc, \
             tc.tile_pool(name="gat", bufs=16) as gat, \
             tc.tile_pool(name="ps_bc", bufs=1, space="PSUM") as ps_bc:
            wih = bc.tile([128, 2, 3, G3], BF16)   # [kpart, dir, kchunk, gcol]
            for d in range(2):
                for k, (k0, kn) in enumerate(KCH):
                    nc.sync.dma_start(out=wih[:kn, d, k, :],
                                      in_=wih_d[d, k0:k0 + kn, :])
            eT = bc.tile([128, 3, NTOK], BF16)     # 24 KB/part

            def xproj_chunk(d, gt, ch):
                px = ps_bc.tile([128, 512], F32, tag="px", bufs=4)
                for k, (k0, kn) in enumerate(KCH):
                    nc.tensor.matmul(
                        px[:, :],
                        lhsT=wih[:kn, d, k, gt * H:(gt + 1) * H],
                        rhs=eT[:kn, k, ch * 512:(ch + 1) * 512],
                        start=(k == 0), stop=(k == 2))
                src = _v(px, [[BL, 32], [1, BL]])
                if gt < 2:
                    blk = _BLKRZ[(d, gt)]
                    dst = _sub(xprz[:], (WU + ch * 32) * RZW + blk * BL,
                               [[RZW, 32], [1, BL]])
                    bcol = blk
                else:
                    dst = _sub(xpn[:], (WU + ch * 32) * NW + d * BL,
                               [[NW, 32], [1, BL]])
                    bcol = 4 + d
                eng = nc.vector if (d * 3 + gt + ch) % 2 == 0 else nc.scalar
                if zero_bx:
                    if eng is nc.vector:
                        eng.tensor_copy(dst, src)
                    else:
                        eng.copy(dst, src)
                else:
                    if eng is nc.vector:
                        eng.tensor_scalar_add(dst, src, biasx[:, bcol:bcol + 1])
                    else:
                        eng.activation(dst, src, AF.Identity,
                                       bias=biasx[:, bcol:bcol + 1])

            for i in range(NGRP):
                g = gat.tile([128, D_W], BF16, name="g", tag="g")
                nc.gpsimd.indirect_dma_start(
                    out=g[:], out_offset=None,
                    in_=emb_d[:, :],
                    in_offset=IndirectOffsetOnAxis(ap=xidx[:, i:i + 1],
                                                   axis=0))
                pt = ps_bc.tile([128, 3, 128], BF16, tag="ptr", bufs=4)
                for k, (k0, kn) in enumerate(KCH):
                    nc.tensor.transpose(pt[:kn, k, :], g[:, k0:k0 + kn],
                                        identb[:])
                # evacuate: chunks 0,1 full-partition; chunk 2 is 44 rows
                e01 = _sub(eT[:], i * 128, [[NTOK, 2], [1, 128]])
                if i % 2 == 0:
                    nc.vector.tensor_copy(e01, pt[:, 0:2, :])
                    nc.scalar.copy(eT[:44, 2, i * 128:(i + 1) * 128],
                                   pt[:44, 2, :])
                else:
                    nc.scalar.copy(e01, pt[:, 0:2, :])
                    nc.vector.tensor_copy(eT[:44, 2, i * 128:(i + 1) * 128],
                                          pt[:44, 2, :])
                # interleave x_proj for completed 512-token chunks so the PE
                # streams matmuls while later groups are still gathering
                if i % 4 == 3:
                    ch = i // 4
                    for d in range(2):
                        for gt in range(3):
                            xproj_chunk(d, gt, ch)

        # ---- phase D: chunked-parallel scan ----
        # PCH chunks per direction run concurrently, batched into single wide
        # instructions; WU warmup slots rebuild each chunk's entry state via
        # GRU forgetting (the -30/30 xp pads handle the sequence edges).
        # The fp16 state lives directly in hbf: step k reads the slots step
        # k-1 wrote; warmup writes land in the previous/next chunk's range
        # and are overwritten later by that chunk's true values.
        CHD = [[CCH * BL, PCH], [1, BL]]
        # zero each chunk's first read slot (f: c*CCH ; b: (c+1)*CCH+WU+1)
        nc.vector.memset(_sub(hbf[:], 0, CHD), 0.0)
        nc.gpsimd.memset(_sub(hbf[:], (HB0 + (CCH + WU + 1) * BL), CHD), 0.0)
        with tc.tile_pool(name="ps_scan", bufs=1, space="PSUM") as ps_sc:
            for k in range(WU + CCH):
                # emit op-type-major so each engine's program order alternates
                # directions -- otherwise in-order engines serialize the two
                # independent per-direction dependency chains
                st = [dict() for _ in range(2)]
                for d in range(2):
                    s = st[d]
                    s["prz"] = ps_sc.tile([128, 2 * PB], F32, tag=f"prz{d}",
                                          bufs=2, name=f"prz{d}")
                    s["pn"] = ps_sc.tile([128, PB], F32, tag=f"pn{d}", bufs=2,
                                         name=f"pn{d}")
                    xo = k if d == 0 else (CCH - 1 + 2 * WU - k)
                    s["xr"] = _sub(xprz[:],
                                   xo * RZW + (2 * BL if d == 1 else 0),
                                   [[BL, 2], [CCH * RZW, PCH], [1, BL]])
                    s["xn"] = _sub(xpn[:], xo * NW + d * BL,
                                   [[CCH * NW, PCH], [1, BL]])
                    if d == 0:
                        s["h_rd"] = _sub(hbf[:], k * BL, CHD)
                        s["h_wr"] = _sub(hbf[:], (k + 1) * BL, CHD)
                    else:
                        s["h_rd"] = _sub(hbf[:],
                                         HB0 + (CCH + WU + 1 - k) * BL, CHD)
                        s["h_wr"] = _sub(hbf[:], HB0 + (CCH + WU - k) * BL,
                                         CHD)
                for d in range(2):
                    s = st[d]
                    # r and z as independent accumulation groups in one bank
                    # so sig_r fires as soon as g_r lands (not after g_z)
                    xr_r = bass.AP(tensor=s["xr"].tensor, offset=s["xr"].offset,
                                   ap=[s["xr"].ap[0]] + s["xr"].ap[2:])
                    xr_z = bass.AP(tensor=s["xr"].tensor,
                                   offset=s["xr"].offset + BL,
                                   ap=[s["xr"].ap[0]] + s["xr"].ap[2:])
                    mi_r = nc.tensor.matmul(s["prz"][:, 0:PB], lhsT=identb[:],
                                            rhs=xr_r, start=True, stop=False)
                    g_r = nc.tensor.matmul(s["prz"][:, 0:PB],
                                           lhsT=whh[:, d, 0:H], rhs=s["h_rd"],
                                           start=False, stop=True)
                    add_dep_helper(g_r.ins, mi_r.ins, sync=False, reason="acc")
                    mi_z = nc.tensor.matmul(s["prz"][:, PB:2 * PB],
                                            lhsT=identb[:], rhs=xr_z,
                                            start=True, stop=False)
                    g_z = nc.tensor.matmul(s["prz"][:, PB:2 * PB],
                                           lhsT=whh[:, d, H:2 * H],
                                           rhs=s["h_rd"],
                                           start=False, stop=True)
                    add_dep_helper(g_z.ins, mi_z.ins, sync=False, reason="acc")
                    nc.tensor.matmul(s["pn"][:], lhsT=whh[:, d, 2 * H:3 * H],
                                     rhs=s["h_rd"], start=True, stop=True)
                # r-sigmoid first: it alone gates the tanh path
                for d in range(2):
                    s = st[d]
                    s["rwr"] = work.tile([128, PB], BF16, tag=f"rwr{d}",
                                         name=f"rwr{d}")
                    nc.scalar.activation(s["rwr"][:], s["prz"][:, 0:PB],
                                         AF.Sigmoid)
                for d in range(2):
                    s = st[d]
                    s["rwz"] = work.tile([128, PB], BF16, tag=f"rwz{d}",
                                         name=f"rwz{d}")
                    nc.scalar.activation(s["rwz"][:], s["prz"][:, PB:2 * PB],
                                         AF.Sigmoid)
                # u = (1-w)*h off the critical chain (needs only w and h)
                for d in range(2):
                    s = st[d]
                    s["a"] = work.tile([128, PB], BF16, tag=f"a{d}",
                                       name=f"a{d}")
                    nc.gpsimd.tensor_tensor(
                        _v(s["a"], [[BL, PCH], [1, BL]]),
                        _v(s["rwz"], [[BL, PCH], [1, BL]]), s["h_rd"],
                        op=OP.mult)
                for d in range(2):
                    s = st[d]
                    s["u"] = work.tile([128, PB], BF16, tag=f"u{d}",
                                       name=f"u{d}")
                    nc.gpsimd.tensor_tensor(
                        _v(s["u"], [[BL, PCH], [1, BL]]), s["h_rd"],
                        _v(s["a"], [[BL, PCH], [1, BL]]), op=OP.subtract)
                for d in range(2):
                    s = st[d]
                    s["tn"] = work.tile([128, PB], BF16, tag=f"tn{d}",
                                        name=f"tn{d}")
                    if zero_bhn:
                        nc.vector.tensor_tensor(s["tn"][:], s["pn"][:],
                                                s["rwr"][:], op=OP.mult)
                    else:
                        nc.vector.scalar_tensor_tensor(
                            s["tn"][:], s["pn"][:], bhn[:, d:d + 1],
                            s["rwr"][:], op0=OP.add, op1=OP.mult)
                    # t2 immediately follows on DVE: in-order, no sem hop
                    s["t2"] = work.tile([128, PB], BF16, tag=f"t2{d}",
                                        name=f"t2{d}")
                    nc.vector.tensor_tensor(
                        _v(s["t2"], [[BL, PCH], [1, BL]]),
                        _v(s["tn"], [[BL, PCH], [1, BL]]), s["xn"], op=OP.add)
                for d in range(2):
                    s = st[d]
                    s["n"] = work.tile([128, PB], BF16, tag=f"n{d}",
                                       name=f"n{d}")
                    nc.scalar.activation(s["n"][:], s["t2"][:], AF.Tanh)
                # h' = w*n - (w-1)*h ; v then h' back-to-back on DVE
                for d in range(2):
                    s = st[d]
                    s["v"] = work.tile([128, PB], BF16, tag=f"v{d}",
                                       name=f"v{d}")
                    nc.vector.tensor_tensor(s["v"][:], s["rwz"][:], s["n"][:],
                                            op=OP.mult)
                    nc.vector.tensor_tensor(s["h_wr"],
                                            _v(s["v"], [[BL, PCH], [1, BL]]),
                                            _v(s["u"], [[BL, PCH], [1, BL]]),
                                            op=OP.add)

        if debug:
            nc.sync.dma_start(out=dbg_hs_d[:, :], in_=hbf[:])

        # ---- phases E/F/G ----
        with tc.tile_pool(name="ef", bufs=1) as ef, \
             tc.tile_pool(name="rp", bufs=1) as rp, \
             tc.tile_pool(name="ps_ef", bufs=1, space="PSUM") as ps_ef:
            # capsule u_hat, stored f16 in native (group, i, k) order
            uh = ef.tile([128, NGRP * 160], BF16)
            for c2 in range(NGRP // 2):
                pu = ps_ef.tile([128, 2, 160], F32, tag="pu", bufs=2)
                for j in range(2):
                    c = 2 * c2 + j
                    lhs_f = _sub(hbf[:], (WU + 1 + 8 * c) * BL, [[1, 128]])
                    lhs_b = _sub(hbf[:], HB0 + (1 + 8 * c) * BL, [[1, 128]])
                    nc.tensor.matmul(pu[:, j, :], lhsT=lhs_f,
                                     rhs=wcap[:, 0, :], start=True, stop=False)
                    nc.tensor.matmul(pu[:, j, :], lhsT=lhs_b,
                                     rhs=wcap[:, 1, :], start=False, stop=True)
                dst = uh[:, c2 * 320:(c2 + 1) * 320]
                if c2 % 2 == 0:
                    nc.vector.tensor_copy(dst, pu[:, :, :])
                else:
                    nc.scalar.copy(dst, pu[:, :, :])

            # routing state
            c_t = rp.tile([128, NGRP * NUM_CAP], BF16, tag="c")  # [p, g, i]
            bl_t = rp.tile([128, NGRP * NUM_CAP], F32, tag="bl")
            du_t = rp.tile([128, NGRP * NUM_CAP], F32, tag="du")
            outputs = rp.tile([BL, 160], F32, tag="outs")        # (k,i)
            tmp = rp.tile([128, NGRP * 160], BF16, tag="tmp")
            tmp2 = rp.tile([128, NGRP * 160], BF16, tag="tmp2")

            PIECES = [(0, 11), (11, POOL_GRP), (11 + POOL_GRP,
                                                NGRP - 11 - POOL_GRP)]
            # tmp2 split favors GpSimd: DVE also owns the du reduces
            PIECES_O = [(0, 9), (9, 14), (23, 9)]

            def big_mult(dst_t, in1_spec, pieces=PIECES):
                """dst = uh * broadcast(in1) over group ranges."""
                for pi, (lo, cnt) in enumerate(pieces):
                    eng = nc.gpsimd if pi == 1 else nc.vector
                    in1, d02 = in1_spec(lo, cnt)
                    dims = ([[160, cnt], [DIM_CAP, NUM_CAP], [1, DIM_CAP]]
                            if d02 else [[160, cnt], [1, 160]])
                    eng.tensor_tensor(
                        _sub(dst_t[:], lo * 160, dims),
                        _sub(uh[:], lo * 160, dims),
                        in1, op=OP.mult)

            for it in range(ROUTINGS):
                last = it == ROUTINGS - 1
                if it > 0:
                    # softmax over capsules (free groups of NUM_CAP)
                    sb_t = rp.tile([128, NGRP * NUM_CAP], F32, tag="sb",
                                   bufs=2)
                    nc.scalar.activation(sb_t[:], bl_t[:], AF.Exp)
                    sm = rp.tile([128, NGRP], F32, tag="sm", bufs=2)
                    nc.vector.tensor_reduce(
                        sm[:], _v(sb_t, [[NUM_CAP, NGRP], [1, NUM_CAP]]),
                        axis=AX.X, op=OP.add)
                    rc = rp.tile([128, NGRP], F32, tag="rc", bufs=2)
                    nc.vector.reciprocal(rc[:], sm[:])
                    nc.vector.tensor_tensor(
                        _v(c_t, [[NUM_CAP, NGRP], [1, NUM_CAP]]),
                        _v(sb_t, [[NUM_CAP, NGRP], [1, NUM_CAP]]),
                        _v(rc, [[1, NGRP], [0, NUM_CAP]]), op=OP.mult)

                    # tmp = u_hat * c (c broadcast over k)
                    big_mult(tmp, lambda lo, cnt: (
                        _sub(c_t[:], lo * NUM_CAP,
                             [[NUM_CAP, cnt], [1, NUM_CAP], [0, DIM_CAP]]),
                        True))
                    mm_src = tmp[:]
                else:
                    mm_src = uh[:]  # c uniform: scale cancels in squash

                po = ps_ef.tile([BL, 160], F32, tag="po", bufs=2)
                for j in range(NGRP):
                    nc.tensor.matmul(po[:], lhsT=selB[:],
                                     rhs=_sub(mm_src, j * 160, [[1, 160]]),
                                     start=(j == 0), stop=(j == NGRP - 1))
                # squash scale via exp(-0.5*ln(ssum+eps)); for it<4 the
                # normalization is deferred to the per-capsule du scale so
                # the u_hat*po pass starts without waiting on the sqrt chain
                if not last:
                    # broadcast path first: it gates the big tmp2 pass
                    poe = rp.tile([BL, 176], BF16, tag="poe", bufs=2)
                    nc.scalar.copy(poe[:, 0:160], po[:])
                sq = rp.tile([BL, 160], F32, tag="sq", bufs=2)
                nc.scalar.square(sq[:], po[:])
                ssum = rp.tile([BL, NUM_CAP], F32, tag="ssum", bufs=2)
                nc.vector.tensor_reduce(
                    ssum[:], _v(sq, [[DIM_CAP, NUM_CAP], [1, DIM_CAP]]),
                    axis=AX.X, op=OP.add)
                lns = rp.tile([BL, NUM_CAP], F32, tag="lns", bufs=2)
                nc.scalar.activation(lns[:], ssum[:], AF.Ln,
                                     bias=epst[:BL, 0:1])
                rs = rp.tile([BL, NUM_CAP], F32, tag="rs", bufs=2)
                nc.scalar.activation(rs[:], lns[:], AF.Exp, scale=-0.5)
                if last:
                    nc.vector.tensor_tensor(
                        _v(outputs, [[DIM_CAP, NUM_CAP], [1, DIM_CAP]]),
                        _v(po, [[DIM_CAP, NUM_CAP], [1, DIM_CAP]]),
                        _v(rs, [[1, NUM_CAP], [0, DIM_CAP]]), op=OP.mult)

                if not last:
                    # two broadcast matmuls on disjoint bank regions: the po
                    # part fires immediately (gates tmp2); rs follows the
                    # squash chain independently
                    pobr = ps_ef.tile([128, 176], F32, tag="pobr", bufs=1)
                    nc.tensor.matmul(pobr[:, 0:160], lhsT=selTb[:],
                                     rhs=poe[:, 0:160], start=True, stop=True)
                    obr = rp.tile([128, 176], BF16, tag="obr", bufs=2)
                    nc.vector.tensor_copy(obr[:, 0:160], pobr[:, 0:160])
                    nc.scalar.copy(poe[:, 160:170], rs[:])
                    nc.tensor.matmul(pobr[:, 160:170], lhsT=selTb[:],
                                     rhs=poe[:, 160:170], start=True,
                                     stop=True)
                    nc.vector.tensor_copy(obr[:, 160:170], pobr[:, 160:170])
                    # tmp2 = u_hat * po_bcast (broadcast over groups)
                    big_mult(tmp2, lambda lo, cnt: (
                        _sub(obr[:], 0, [[0, cnt], [1, 160]]), False),
                        pieces=PIECES_O)
                    # du_raw = sum over k ; du = du_raw * rs (deferred norm)
                    # pieces aligned with tmp2 splits so each reduce starts
                    # as soon as its group range is multiplied
                    for lo, cnt in PIECES_O:
                        nc.vector.tensor_reduce(
                            _sub(du_t[:], lo * NUM_CAP,
                                 [[NUM_CAP, cnt], [1, NUM_CAP]]),
                            _sub(tmp2[:], lo * 160,
                                 [[160, cnt], [DIM_CAP, NUM_CAP],
                                  [1, DIM_CAP]]),
                            axis=AX.X, op=OP.add)
                    rsb_ap = _sub(obr[:], 160, [[0, NGRP], [1, NUM_CAP]])
                    if it == 0:
                        nc.vector.tensor_tensor(
                            _v(bl_t, [[NUM_CAP, NGRP], [1, NUM_CAP]]),
                            _v(du_t, [[NUM_CAP, NGRP], [1, NUM_CAP]]),
                            rsb_ap, op=OP.mult)
                    else:
                        dus = rp.tile([128, NGRP * NUM_CAP], F32, tag="dus",
                                      bufs=2)
                        nc.vector.tensor_tensor(
                            _v(dus, [[NUM_CAP, NGRP], [1, NUM_CAP]]),
                            _v(du_t, [[NUM_CAP, NGRP], [1, NUM_CAP]]),
                            rsb_ap, op=OP.mult)
                        nc.gpsimd.tensor_add(bl_t[:], bl_t[:], dus[:])

            # final linear (wlin is host-permuted to (k,i) row order)
            pt1 = ps_ef.tile([128, BL], F32, tag="pt1", bufs=1)
            nc.tensor.matmul(pt1[:, :], lhsT=outputs[:, 0:128],
                             rhs=ident[:BL, :BL], start=True, stop=True)
            pt2 = ps_ef.tile([32, BL], F32, tag="pt2", bufs=1)
            nc.tensor.matmul(pt2[:, :], lhsT=outputs[:, 128:160],
                             rhs=ident[:BL, :BL], start=True, stop=True)
            capsT = rp.tile([128, 2 * BL], F32, tag="capsT")
            nc.vector.tensor_copy(capsT[:, 0:BL], pt1[:])
            nc.vector.tensor_copy(capsT[:32, BL:2 * BL], pt2[:])
            pf = ps_ef.tile([2, BL], F32, tag="pf", bufs=1)
            nc.tensor.matmul(pf[:], lhsT=wlin[:, 0, :], rhs=capsT[:, 0:BL],
                             start=True, stop=False)
            nc.tensor.matmul(pf[:], lhsT=wlin[:32, 1, :],
                             rhs=capsT[:32, BL:2 * BL],
                             start=False, stop=True)
            outT = rp.tile([2, BL], F32, tag="outT")
            nc.scalar.activation(outT[:], pf[:], AF.Identity,
                                 bias=blin[:, 0:1])
            dst = bass.AP(tensor=out_d, offset=0, ap=[[1, 2], [2, BL]])
            # issue from the Scalar queue: the Sync queue drains a long
            # event-semaphore backlog at kernel end (~9us) and would delay
            # this DMA; Scalar also produced outT, so this issues in-order
            nc.scalar.dma_start(out=dst, in_=outT[:])

    return nc


_CACHE = {}


def _get_nc(zero_bhn, zero_bx):
    key = (zero_bhn, zero_bx)
    if key not in _CACHE:
        nc = _build(zero_bhn, zero_bx)
        _split_waits(nc)   # HW-path legalization (CoreSim path builds its own)
        _CACHE[key] = nc
    return _CACHE[key]


def _host_inputs(x, emb, w_ih_f, w_hh_f, b_ih_f, b_hh_f,
                 w_ih_b, w_hh_b, b_ih_b, b_hh_b, W_cap, W_lin, b_lin):
    """Build the per-core input maps (everything but xidx is shared)."""
    f32 = np.float32
    bf16 = np.float16
    neg = np.ones((G3,), f32)
    neg[H:2 * H] = -1.0        # negate z gate (sigmoid -> 1-z)

    wih = np.stack([(w_ih_f.T * neg).astype(bf16), (w_ih_b.T * neg).astype(bf16)])
    whh = np.stack([(w_hh_f.T * neg).astype(bf16), (w_hh_b.T * neg).astype(bf16)])

    biasx = np.zeros((128, 6), f32)
    for d, (bi, bh) in enumerate([(b_ih_f, b_hh_f), (b_ih_b, b_hh_b)]):
        biasx[:, _BLKRZ[(d, 0)]] = (bi[0:H] + bh[0:H])
        biasx[:, _BLKRZ[(d, 1)]] = -(bi[H:2 * H] + bh[H:2 * H])
        biasx[:, 4 + d] = bi[2 * H:3 * H]
    bhn = np.zeros((128, 2), f32)
    bhn[:, 0] = b_hh_f[2 * H:3 * H]
    bhn[:, 1] = b_hh_b[2 * H:3 * H]
    zero_bhn = bool(np.all(bhn == 0.0))
    zero_bx = bool(np.all(biasx == 0.0))

    wcap = np.stack([W_cap[0:H, :].astype(bf16), W_cap[H:2 * H, :].astype(bf16)])
    selB = (np.arange(128)[:, None] % BL == np.arange(BL)[None, :]).astype(f32)
    selT = selB.T.copy()
    ident = np.eye(128, dtype=f32)

    shared = dict(emb=np.ascontiguousarray(emb).astype(bf16), wih=wih, whh=whh,
                  biasx=biasx, bhn=bhn, wcap=wcap,
                  wlin=np.ascontiguousarray(W_lin, f32),
                  blin=np.ascontiguousarray(b_lin, f32).reshape(2, 1),
                  selB=selB, selT=selT, ident=ident)

    in_maps = []
    for c in range(NCORES):
        xl = np.asarray(x[c * BL:(c + 1) * BL, :])          # [BL, S]
        tok = xl.T.reshape(-1).astype(np.int32)             # s-major [NTOK]
        xidx = np.ascontiguousarray(tok.reshape(NGRP, 128).T)  # [128, NGRP]
        in_maps.append(dict(shared, xidx=xidx))
    return in_maps, zero_bhn, zero_bx


def kernel(**inputs):
    in_maps, zero_bhn, zero_bx = _host_inputs(
        **{k: np.asarray(v) for k, v in inputs.items()})
    nc = _get_nc(zero_bhn, zero_bx)
    res = run_bass_kernel_spmd(nc, in_maps, list(range(NCORES)))
    return np.concatenate([res.results[c]["out"] for c in range(NCORES)],
                          axis=0)


def _install_ntff_hook():
    """Shim the missing antenv.axon_hooks so trace=True works under axon."""
    import sys, types
    if "antenv.axon_hooks" in sys.modules:
        return
    mod = types.ModuleType("antenv.axon_hooks")
    _h = [None]
    mod.set_axon_ntff_profile_hook = lambda h: _h.__setitem__(0, h)
    mod.get_axon_ntff_profile_hook = lambda: _h[0]
    sys.modules["antenv.axon_hooks"] = mod
    import antenv
    antenv.axon_hooks = mod
    from trn_agent_boot.trn_boot import _ntff_profile_via_ctypes
    mod.set_axon_ntff_profile_hook(
        _ntff_profile_via_ctypes("/opt/axon/libaxon_pjrt.so"))


def kernel_profiled(**inputs):
    """Same as kernel() but with NTFF tracing; returns (out, result_obj)."""
    _install_ntff_hook()
    in_maps, zero_bhn, zero_bx = _host_inputs(
        **{k: np.asarray(v) for k, v in inputs.items()})
    nc = _get_nc(zero_bhn, zero_bx)
    res = run_bass_kernel_spmd(nc, in_maps, list(range(NCORES)), trace=True)
    out = np.concatenate([res.results[c]["out"] for c in range(NCORES)],
                         axis=0)
    return out, res


# revision 48
# speedup vs baseline: 1.0317x; 1.0317x over previous
"""Trainium2 Bass kernel for nn_CapRNNModelHelper (bi-GRU + capsule routing).

Sharding: data-parallel over batch across 8 cores (16 batch rows per core).
Everything else (embedding table, GRU weights, capsule weights) replicated.

Per-core pipeline (fp16 matmul operands, f32 accumulation):
  1. indirect-DMA gather of f16 embedding rows (token order s-major)
  2. PE-transpose (f16 psum) -> e.T  [300, ntok] fp16
  3. x_proj matmuls (fp16) -> xp_rz (fp16) + xp_n (f32), biases folded,
     z blocks negated so sigmoid gives w = 1-z directly
  4. 256-step fused bidirectional GRU scan: per step the xp_rz slice is
     PSUM-accumulated via an identity matmul, gates matmul on top (fp16
     weights, fp16 h mirror), sigmoid/tanh on ACT, update on DVE+GpSimd
  5. capsule matmul (fp16) -> u_hat [sb, (k,i)160] f16
  6. 5-iter dynamic routing (selector matmuls for sequence reductions,
     iter-0 coupling fold, exp/ln-based 1/sqrt to stay in one ACT table)
  7. final linear -> out [16, 2]
"""

import numpy as np
from contextlib import ExitStack

import concourse.bass as bass
import concourse.tile as tile
from concourse import mybir
from concourse.bass import IndirectOffsetOnAxis
from concourse.bass_utils import run_bass_kernel_spmd
from concourse.tile_rust import add_dep_helper

F32 = mybir.dt.float32
BF16 = mybir.dt.float16
I32 = mybir.dt.int32
AF = mybir.ActivationFunctionType
OP = mybir.AluOpType
AX = mybir.AxisListType

VOCAB, D_W, H, S, B = 50000, 300, 128, 256, 128
NUM_CAP, DIM_CAP, ROUTINGS, EPS = 10, 16, 5, 1e-7
NCORES = 8
BL = B // NCORES          # 16 batch rows per core
NTOK = S * BL             # 4096 tokens per core
NGRP = NTOK // 128        # 32 gather groups of 128 tokens
NCH = NTOK // 512         # 8 x_proj chunks of 512 tokens
KCH = [(0, 128), (128, 128), (256, 44)]   # D_W split
G3 = 3 * H                # 384
GG = 1                    # token groups per indirect gather

RZW = 4 * BL              # 64   per-step rz width [rf zf rb zb]
NW = 2 * BL               # 32   per-step n width [nf nb]
PCH = 16                  # parallel chunks per direction in the scan
CCH = S // PCH            # 16 steps per chunk
WU = 9                    # warmup steps per chunk (GRU forgetting rebuilds h)
EXT = S + 2 * WU          # padded xp timeline
PB = PCH * BL             # 256  scan op width per direction
FSL = S + WU + 1          # forward h slots (position p at slot p+WU+1)
BSL = S + WU + 2          # backward h slots (position p at slot p+1)
# block index for (dir d, gate g): rz blocks 0..3, n blocks 0..1
_BLKRZ = {(0, 0): 0, (0, 1): 1, (1, 0): 2, (1, 1): 3}

# routing: groups handled by DVE vs GpSimd on the big elementwise ops
POOL_GRP = 10             # groups on GpSimd in big-op splits
DVE_GRP = NGRP - POOL_GRP


def _sub(base, off, dims):
    """Manual AP: base is a [128, X] AP; append free dims after partition."""
    return bass.AP(tensor=base.tensor, offset=base.offset + off,
                   ap=[base.ap[0]] + dims)


def _v(t, dims, off=0):
    return bass.AP(tensor=t.tensor, offset=t.offset + off,
                   ap=[t.ap[0]] + dims)


def _split_waits(nc, cap=1):
    """Hoist excess sync waits onto standalone event-semaphore ops.

    The walrus build on this stack accepts only `cap` sync-wait commands
    per ISA instruction; Tile can attach several. Event-semaphore ops on
    the same engine execute in queue order, so hoisting preserves
    semantics.
    """
    n = 0
    for fn in nc.m.functions:
        for bb in fn.blocks:
            out = []
            for ins in bb.instructions:
                si = ins.sync_info
                if si is not None and len(si.on_wait) > cap:
                    waits = list(si.on_wait)
                    keep = waits[len(waits) - cap:] if cap else []
                    for w in waits[:len(waits) - cap] if cap else waits:
                        n += 1
                        out.append(mybir.InstEventSemaphore(
                            name=f"wsplit-{n}", engine=ins.engine,
                            ins=[], outs=[],
                            sync_info=mybir.SyncInfo(on_wait=[w],
                                                     on_update=[])))
                    ins.sync_info = mybir.SyncInfo(
                        on_wait=keep, on_update=list(si.on_update))
                out.append(ins)
            bb.instructions = out
    return n


def _build(zero_bhn: bool, zero_bx: bool, debug: bool = False):
    nc = bass.Bass()
    if debug:
        dbg_hs_d = nc.declare_dram_parameter(
            "dbg_hs", [128, (FSL + BSL) * BL], BF16, True)

    xidx_d = nc.declare_dram_parameter("xidx", [128, NGRP], I32, False)
    emb_d = nc.declare_dram_parameter("emb", [VOCAB, D_W], BF16, False)
    wih_d = nc.declare_dram_parameter("wih", [2, D_W, G3], BF16, False)
    whh_d = nc.declare_dram_parameter("whh", [2, H, G3], BF16, False)
    biasx_d = nc.declare_dram_parameter("biasx", [128, 6], F32, False)
    bhn_d = nc.declare_dram_parameter("bhn", [128, 2], F32, False)
    wcap_d = nc.declare_dram_parameter("wcap", [2, H, 160], BF16, False)
    wlin_d = nc.declare_dram_parameter("wlin", [160, 2], F32, False)
    blin_d = nc.declare_dram_parameter("blin", [2, 1], F32, False)
    selB_d = nc.declare_dram_parameter("selB", [128, BL], F32, False)
    selT_d = nc.declare_dram_parameter("selT", [BL, 128], F32, False)
    ident_d = nc.declare_dram_parameter("ident", [128, 128], F32, False)
    out_d = nc.declare_dram_parameter("out", [BL, 2], F32, True)

    with tile.TileContext(nc) as tc, ExitStack() as ctx:
        const = ctx.enter_context(tc.tile_pool(name="const", bufs=1))
        bigxp = ctx.enter_context(tc.tile_pool(name="bigxp", bufs=1))
        bighs = ctx.enter_context(tc.tile_pool(name="bighs", bufs=1))
        work = ctx.enter_context(tc.tile_pool(name="work", bufs=3))

        # ---- constants to SBUF ----
        xidx = const.tile([128, NGRP], I32)
        # scalar HWDGE ring: empty at start, so the gathers unblock sooner
        nc.scalar.dma_start(out=xidx[:], in_=xidx_d[:, :])
        whh = const.tile([128, 2, G3], BF16)
        for d in range(2):
            nc.sync.dma_start(out=whh[:, d, :], in_=whh_d[d, :, :])
        biasx = const.tile([128, 6], F32)
        nc.sync.dma_start(out=biasx[:], in_=biasx_d[:, :])
        bhn = const.tile([128, 2], F32)
        nc.sync.dma_start(out=bhn[:], in_=bhn_d[:, :])
        wcap = const.tile([128, 2, 160], BF16)
        for k in range(2):
            nc.sync.dma_start(out=wcap[:, k, :], in_=wcap_d[k, :, :])
        wlin = const.tile([128, 2, 2], F32)        # chunk0 [:128], chunk1 [:32]
        nc.sync.dma_start(out=wlin[:, 0, :], in_=wlin_d[0:128, :])
        nc.sync.dma_start(out=wlin[:32, 1, :], in_=wlin_d[128:160, :])
        blin = const.tile([2, 1], F32)
        nc.sync.dma_start(out=blin[:], in_=blin_d[:, :])
        selBf = const.tile([128, BL], F32)
        nc.sync.dma_start(out=selBf[:], in_=selB_d[:, :])
        selB = const.tile([128, BL], BF16)
        nc.scalar.copy(selB[:], selBf[:])
        selT = const.tile([BL, 128], F32)
        nc.sync.dma_start(out=selT[:], in_=selT_d[:, :])
        selTb = const.tile([BL, 128], BF16)
        nc.scalar.copy(selTb[:], selT[:])
        ident = const.tile([128, 128], F32)
        nc.sync.dma_start(out=ident[:], in_=ident_d[:, :])
        identb = const.tile([128, 128], BF16)
        nc.scalar.copy(identb[:], ident[:])
        epst = const.tile([128, 1], F32)
        nc.vector.memset(epst[:], EPS)

        xprz = bigxp.tile([128, EXT * RZW], BF16)   # 35 KB/part
        xpn = bigxp.tile([128, EXT * NW], BF16)     # 17.5 KB/part
        HB0 = FSL * BL                              # backward region base
        hbf = bighs.tile([128, (FSL + BSL) * BL], BF16)  # 17.2 KB/part
        # warmup pads force h -> 0 exactly: r=sigmoid(-30)=0, w=sigmoid(30)=1,
        # xn=0  =>  h' = 1*tanh(0) + 0*h = 0
        for p0 in (0, S + WU):
            for blk, val in ((0, -30.0), (1, 30.0), (2, -30.0), (3, 30.0)):
                nc.vector.memset(_sub(xprz[:], p0 * RZW + blk * BL,
                                      [[RZW, WU], [1, BL]]), val)
            nc.gpsimd.memset(_sub(xpn[:], p0 * NW, [[1, WU * NW]]), 0.0)

        # ---- phases B+C: gather + transpose + x_proj (single pass) ----
        with tc.tile_pool(name="bc", bufs=1) as bc, \
             tc.tile_pool(name="gat", bufs=16) as gat, \
             tc.tile_pool(name="ps_bc", bufs=1, space="PSUM") as ps_bc:
            wih = bc.tile([128, 2, 3, G3], BF16)   # [kpart, dir, kchunk, gcol]
            for d in range(2):
                for k, (k0, kn) in enumerate(KCH):
                    nc.sync.dma_start(out=wih[:kn, d, k, :],
                                      in_=wih_d[d, k0:k0 + kn, :])
            eT = bc.tile([128, 3, NTOK], BF16)     # 24 KB/part

            def xproj_chunk(d, gt, ch):
                px = ps_bc.tile([128, 512], F32, tag="px", bufs=4)
                for k, (k0, kn) in enumerate(KCH):
                    nc.tensor.matmul(
                        px[:, :],
                        lhsT=wih[:kn, d, k, gt * H:(gt + 1) * H],
                        rhs=eT[:kn, k, ch * 512:(ch + 1) * 512],
                        start=(k == 0), stop=(k == 2))
                src = _v(px, [[BL, 32], [1, BL]])
                if gt < 2:
                    blk = _BLKRZ[(d, gt)]
                    dst = _sub(xprz[:], (WU + ch * 32) * RZW + blk * BL,
                               [[RZW, 32], [1, BL]])
                    bcol = blk
                else:
                    dst = _sub(xpn[:], (WU + ch * 32) * NW + d * BL,
                               [[NW, 32], [1, BL]])
                    bcol = 4 + d
                eng = nc.vector if (d * 3 + gt + ch) % 2 == 0 else nc.scalar
                if zero_bx:
                    if eng is nc.vector:
                        eng.tensor_copy(dst, src)
                    else:
                        eng.copy(dst, src)
                else:
                    if eng is nc.vector:
                        eng.tensor_scalar_add(dst, src, biasx[:, bcol:bcol + 1])
                    else:
                        eng.activation(dst, src, AF.Identity,
                                       bias=biasx[:, bcol:bcol + 1])

            for i in range(NGRP):
                g = gat.tile([128, D_W], BF16, name="g", tag="g")
                nc.gpsimd.indirect_dma_start(
                    out=g[:], out_offset=None,
                    in_=emb_d[:, :],
                    in_offset=IndirectOffsetOnAxis(ap=xidx[:, i:i + 1],
                                                   axis=0))
                pt = ps_bc.tile([128, 3, 128], BF16, tag="ptr", bufs=4)
                for k, (k0, kn) in enumerate(KCH):
                    nc.tensor.transpose(pt[:kn, k, :], g[:, k0:k0 + kn],
                                        identb[:])
                # evacuate: chunks 0,1 full-partition; chunk 2 is 44 rows
                e01 = _sub(eT[:], i * 128, [[NTOK, 2], [1, 128]])
                if i % 2 == 0:
                    nc.vector.tensor_copy(e01, pt[:, 0:2, :])
                    nc.scalar.copy(eT[:44, 2, i * 128:(i + 1) * 128],
                                   pt[:44, 2, :])
                else:
                    nc.scalar.copy(e01, pt[:, 0:2, :])
                    nc.vector.tensor_copy(eT[:44, 2, i * 128:(i + 1) * 128],
                                          pt[:44, 2, :])
                # interleave x_proj for completed 512-token chunks so the PE
                # streams matmuls while later groups are still gathering
                if i % 4 == 3:
                    ch = i // 4
                    for d in range(2):
                        for gt in range(3):
                            xproj_chunk(d, gt, ch)

        # ---- phase D: chunked-parallel scan ----
        # PCH chunks per direction run concurrently, batched into single wide
        # instructions; WU warmup slots rebuild each chunk's entry state via
        # GRU forgetting (the -30/30 xp pads handle the sequence edges).
        # The fp16 state lives directly in hbf: step k reads the slots step
        # k-1 wrote; warmup writes land in the previous/next chunk's range
        # and are overwritten later by that chunk's true values.
        CHD = [[CCH * BL, PCH], [1, BL]]
        # zero each chunk's first read slot (f: c*CCH ; b: (c+1)*CCH+WU+1)
        nc.vector.memset(_sub(hbf[:], 0, CHD), 0.0)
        nc.gpsimd.memset(_sub(hbf[:], (HB0 + (CCH + WU + 1) * BL), CHD), 0.0)
        with tc.tile_pool(name="ps_scan", bufs=1, space="PSUM") as ps_sc:
            for k in range(WU + CCH):
                # emit op-type-major so each engine's program order alternates
                # directions -- otherwise in-order engines serialize the two
                # independent per-direction dependency chains
                st = [dict() for _ in range(2)]
                for d in range(2):
                    s = st[d]
                    s["prz"] = ps_sc.tile([128, 2 * PB], F32, tag=f"prz{d}",
                                          bufs=2, name=f"prz{d}")
                    s["pn"] = ps_sc.tile([128, PB], F32, tag=f"pn{d}", bufs=2,
                                         name=f"pn{d}")
                    xo = k if d == 0 else (CCH - 1 + 2 * WU - k)
                    s["xr"] = _sub(xprz[:],
                                   xo * RZW + (2 * BL if d == 1 else 0),
                                   [[BL, 2], [CCH * RZW, PCH], [1, BL]])
                    s["xn"] = _sub(xpn[:], xo * NW + d * BL,
                                   [[CCH * NW, PCH], [1, BL]])
                    if d == 0:
                        s["h_rd"] = _sub(hbf[:], k * BL, CHD)
                        s["h_wr"] = _sub(hbf[:], (k + 1) * BL, CHD)
                    else:
                        s["h_rd"] = _sub(hbf[:],
                                         HB0 + (CCH + WU + 1 - k) * BL, CHD)
                        s["h_wr"] = _sub(hbf[:], HB0 + (CCH + WU - k) * BL,
                                         CHD)
                for d in range(2):
                    s = st[d]
                    mi = nc.tensor.matmul(s["prz"][:], lhsT=identb[:],
                                          rhs=s["xr"], start=True, stop=False)
                    g_r = nc.tensor.matmul(s["prz"][:, 0:PB],
                                           lhsT=whh[:, d, 0:H], rhs=s["h_rd"],
                                           start=False, stop=False)
                    add_dep_helper(g_r.ins, mi.ins, sync=False, reason="acc")
                    g_z = nc.tensor.matmul(s["prz"][:, PB:2 * PB],
                                           lhsT=whh[:, d, H:2 * H],
                                           rhs=s["h_rd"],
                                           start=False, stop=True)
                    add_dep_helper(g_z.ins, g_r.ins, sync=False, reason="acc")
                    nc.tensor.matmul(s["pn"][:], lhsT=whh[:, d, 2 * H:3 * H],
                                     rhs=s["h_rd"], start=True, stop=True)
                # r-sigmoid first: it alone gates the tanh path
                for d in range(2):
                    s = st[d]
                    s["rwr"] = work.tile([128, PB], BF16, tag=f"rwr{d}",
                                         name=f"rwr{d}")
                    nc.scalar.activation(s["rwr"][:], s["prz"][:, 0:PB],
                                         AF.Sigmoid)
                for d in range(2):
                    s = st[d]
                    s["rwz"] = work.tile([128, PB], BF16, tag=f"rwz{d}",
                                         name=f"rwz{d}")
                    nc.scalar.activation(s["rwz"][:], s["prz"][:, PB:2 * PB],
                                         AF.Sigmoid)
                # u = (1-w)*h off the critical chain (needs only w and h)
                for d in range(2):
                    s = st[d]
                    s["a"] = work.tile([128, PB], BF16, tag=f"a{d}",
                                       name=f"a{d}")
                    nc.gpsimd.tensor_tensor(
                        _v(s["a"], [[BL, PCH], [1, BL]]),
                        _v(s["rwz"], [[BL, PCH], [1, BL]]), s["h_rd"],
                        op=OP.mult)
                for d in range(2):
                    s = st[d]
                    s["u"] = work.tile([128, PB], BF16, tag=f"u{d}",
                                       name=f"u{d}")
                    nc.gpsimd.tensor_tensor(
                        _v(s["u"], [[BL, PCH], [1, BL]]), s["h_rd"],
                        _v(s["a"], [[BL, PCH], [1, BL]]), op=OP.subtract)
                for d in range(2):
                    s = st[d]
                    s["tn"] = work.tile([128, PB], BF16, tag=f"tn{d}",
                                        name=f"tn{d}")
                    if zero_bhn:
                        nc.vector.tensor_tensor(s["tn"][:], s["pn"][:],
                                                s["rwr"][:], op=OP.mult)
                    else:
                        nc.vector.scalar_tensor_tensor(
                            s["tn"][:], s["pn"][:], bhn[:, d:d + 1],
                            s["rwr"][:], op0=OP.add, op1=OP.mult)
                    # t2 immediately follows on DVE: in-order, no sem hop
                    s["t2"] = work.tile([128, PB], BF16, tag=f"t2{d}",
                                        name=f"t2{d}")
                    nc.vector.tensor_tensor(
                        _v(s["t2"], [[BL, PCH], [1, BL]]),
                        _v(s["tn"], [[BL, PCH], [1, BL]]), s["xn"], op=OP.add)
                for d in range(2):
                    s = st[d]
                    s["n"] = work.tile([128, PB], BF16, tag=f"n{d}",
                                       name=f"n{d}")
                    nc.scalar.activation(s["n"][:], s["t2"][:], AF.Tanh)
                # h' = w*n - (w-1)*h ; v then h' back-to-back on DVE
                for d in range(2):
                    s = st[d]
                    s["v"] = work.tile([128, PB], BF16, tag=f"v{d}",
                                       name=f"v{d}")
                    nc.vector.tensor_tensor(s["v"][:], s["rwz"][:], s["n"][:],
                                            op=OP.mult)
                    nc.vector.tensor_tensor(s["h_wr"],
                                            _v(s["v"], [[BL, PCH], [1, BL]]),
                                            _v(s["u"], [[BL, PCH], [1, BL]]),
                                            op=OP.add)

        if debug:
            nc.sync.dma_start(out=dbg_hs_d[:, :], in_=hbf[:])

        # ---- phases E/F/G ----
        with tc.tile_pool(name="ef", bufs=1) as ef, \
             tc.tile_pool(name="rp", bufs=1) as rp, \
             tc.tile_pool(name="ps_ef", bufs=1, space="PSUM") as ps_ef:
            # capsule u_hat, stored f16 in native (group, i, k) order
            uh = ef.tile([128, NGRP * 160], BF16)
            for c2 in range(NGRP // 2):
                pu = ps_ef.tile([128, 2, 160], F32, tag="pu", bufs=2)
                for j in range(2):
                    c = 2 * c2 + j
                    lhs_f = _sub(hbf[:], (WU + 1 + 8 * c) * BL, [[1, 128]])
                    lhs_b = _sub(hbf[:], HB0 + (1 + 8 * c) * BL, [[1, 128]])
                    nc.tensor.matmul(pu[:, j, :], lhsT=lhs_f,
                                     rhs=wcap[:, 0, :], start=True, stop=False)
                    nc.tensor.matmul(pu[:, j, :], lhsT=lhs_b,
                                     rhs=wcap[:, 1, :], start=False, stop=True)
                dst = uh[:, c2 * 320:(c2 + 1) * 320]
                if c2 % 2 == 0:
                    nc.vector.tensor_copy(dst, pu[:, :, :])
                else:
                    nc.scalar.copy(dst, pu[:, :, :])

            # routing state
            c_t = rp.tile([128, NGRP * NUM_CAP], BF16, tag="c")  # [p, g, i]
            bl_t = rp.tile([128, NGRP * NUM_CAP], F32, tag="bl")
            du_t = rp.tile([128, NGRP * NUM_CAP], F32, tag="du")
            outputs = rp.tile([BL, 160], F32, tag="outs")        # (k,i)
            tmp = rp.tile([128, NGRP * 160], BF16, tag="tmp")
            tmp2 = rp.tile([128, NGRP * 160], BF16, tag="tmp2")

            PIECES = [(0, 11), (11, POOL_GRP), (11 + POOL_GRP,
                                                NGRP - 11 - POOL_GRP)]
            # tmp2 split favors GpSimd: DVE also owns the du reduces
            PIECES_O = [(0, 9), (9, 14), (23, 9)]

            def big_mult(dst_t, in1_spec, pieces=PIECES):
                """dst = uh * broadcast(in1) over group ranges."""
                for pi, (lo, cnt) in enumerate(pieces):
                    eng = nc.gpsimd if pi == 1 else nc.vector
                    in1, d02 = in1_spec(lo, cnt)
                    dims = ([[160, cnt], [DIM_CAP, NUM_CAP], [1, DIM_CAP]]
                            if d02 else [[160, cnt], [1, 160]])
                    eng.tensor_tensor(
                        _sub(dst_t[:], lo * 160, dims),
                        _sub(uh[:], lo * 160, dims),
                        in1, op=OP.mult)

            for it in range(ROUTINGS):
                last = it == ROUTINGS - 1
                if it > 0:
                    # softmax over capsules (free groups of NUM_CAP)
                    sb_t = rp.tile([128, NGRP * NUM_CAP], F32, tag="sb",
                                   bufs=2)
                    nc.scalar.activation(sb_t[:], bl_t[:], AF.Exp)
                    sm = rp.tile([128, NGRP], F32, tag="sm", bufs=2)
                    nc.vector.tensor_reduce(
                        sm[:], _v(sb_t, [[NUM_CAP, NGRP], [1, NUM_CAP]]),
                        axis=AX.X, op=OP.add)
                    rc = rp.tile([128, NGRP], F32, tag="rc", bufs=2)
                    nc.vector.reciprocal(rc[:], sm[:])
                    nc.vector.tensor_tensor(
                        _v(c_t, [[NUM_CAP, NGRP], [1, NUM_CAP]]),
                        _v(sb_t, [[NUM_CAP, NGRP], [1, NUM_CAP]]),
                        _v(rc, [[1, NGRP], [0, NUM_CAP]]), op=OP.mult)

                    # tmp = u_hat * c (c broadcast over k)
                    big_mult(tmp, lambda lo, cnt: (
                        _sub(c_t[:], lo * NUM_CAP,
                             [[NUM_CAP, cnt], [1, NUM_CAP], [0, DIM_CAP]]),
                        True))
                    mm_src = tmp[:]
                else:
                    mm_src = uh[:]  # c uniform: scale cancels in squash

                po = ps_ef.tile([BL, 160], F32, tag="po", bufs=2)
                for j in range(NGRP):
                    nc.tensor.matmul(po[:], lhsT=selB[:],
                                     rhs=_sub(mm_src, j * 160, [[1, 160]]),
                                     start=(j == 0), stop=(j == NGRP - 1))
                # squash scale via exp(-0.5*ln(ssum+eps)); for it<4 the
                # normalization is deferred to the per-capsule du scale so
                # the u_hat*po pass starts without waiting on the sqrt chain
                if not last:
                    # broadcast path first: it gates the big tmp2 pass
                    poe = rp.tile([BL, 176], BF16, tag="poe", bufs=2)
                    nc.scalar.copy(poe[:, 0:160], po[:])
                sq = rp.tile([BL, 160], F32, tag="sq", bufs=2)
                nc.scalar.square(sq[:], po[:])
                ssum = rp.tile([BL, NUM_CAP], F32, tag="ssum", bufs=2)
                nc.vector.tensor_reduce(
                    ssum[:], _v(sq, [[DIM_CAP, NUM_CAP], [1, DIM_CAP]]),
                    axis=AX.X, op=OP.add)
                lns = rp.tile([BL, NUM_CAP], F32, tag="lns", bufs=2)
                nc.scalar.activation(lns[:], ssum[:], AF.Ln,
                                     bias=epst[:BL, 0:1])
                rs = rp.tile([BL, NUM_CAP], F32, tag="rs", bufs=2)
                nc.scalar.activation(rs[:], lns[:], AF.Exp, scale=-0.5)
                if last:
                    nc.vector.tensor_tensor(
                        _v(outputs, [[DIM_CAP, NUM_CAP], [1, DIM_CAP]]),
                        _v(po, [[DIM_CAP, NUM_CAP], [1, DIM_CAP]]),
                        _v(rs, [[1, NUM_CAP], [0, DIM_CAP]]), op=OP.mult)

                if not last:
                    # two broadcast matmuls on disjoint bank regions: the po
                    # part fires immediately (gates tmp2); rs follows the
                    # squash chain independently
                    pobr = ps_ef.tile([128, 176], F32, tag="pobr", bufs=1)
                    nc.tensor.matmul(pobr[:, 0:160], lhsT=selTb[:],
                                     rhs=poe[:, 0:160], start=True, stop=True)
                    obr = rp.tile([128, 176], BF16, tag="obr", bufs=2)
                    nc.vector.tensor_copy(obr[:, 0:160], pobr[:, 0:160])
                    nc.scalar.copy(poe[:, 160:170], rs[:])
                    nc.tensor.matmul(pobr[:, 160:170], lhsT=selTb[:],
                                     rhs=poe[:, 160:170], start=True,
                                     stop=True)
                    nc.vector.tensor_copy(obr[:, 160:170], pobr[:, 160:170])
                    # tmp2 = u_hat * po_bcast (broadcast over groups)
                    big_mult(tmp2, lambda lo, cnt: (
                        _sub(obr[:], 0, [[0, cnt], [1, 160]]), False),
                        pieces=PIECES_O)
                    # du_raw = sum over k ; du = du_raw * rs (deferred norm)
                    # pieces aligned with tmp2 splits so each reduce starts
                    # as soon as its group range is multiplied
                    for lo, cnt in PIECES_O:
                        nc.vector.tensor_reduce(
                            _sub(du_t[:], lo * NUM_CAP,
                                 [[NUM_CAP, cnt], [1, NUM_CAP]]),
                            _sub(tmp2[:], lo * 160,
                                 [[160, cnt], [DIM_CAP, NUM_CAP],
                                  [1, DIM_CAP]]),
                            axis=AX.X, op=OP.add)
                    rsb_ap = _sub(obr[:], 160, [[0, NGRP], [1, NUM_CAP]])
                    if it == 0:
                        nc.vector.tensor_tensor(
                            _v(bl_t, [[NUM_CAP, NGRP], [1, NUM_CAP]]),
                            _v(du_t, [[NUM_CAP, NGRP], [1, NUM_CAP]]),
                            rsb_ap, op=OP.mult)
                    else:
                        dus = rp.tile([128, NGRP * NUM_CAP], F32, tag="dus",
                                      bufs=2)
                        nc.vector.tensor_tensor(
                            _v(dus, [[NUM_CAP, NGRP], [1, NUM_CAP]]),
                            _v(du_t, [[NUM_CAP, NGRP], [1, NUM_CAP]]),
                            rsb_ap, op=OP.mult)
                        nc.gpsimd.tensor_add(bl_t[:], bl_t[:], dus[:])

            # final linear (wlin is host-permuted to (k,i) row order)
            pt1 = ps_ef.tile([128, BL], F32, tag="pt1", bufs=1)
            nc.tensor.matmul(pt1[:, :], lhsT=outputs[:, 0:128],
                             rhs=ident[:BL, :BL], start=True, stop=True)
            pt2 = ps_ef.tile([32, BL], F32, tag="pt2", bufs=1)
            nc.tensor.matmul(pt2[:, :], lhsT=outputs[:, 128:160],
                             rhs=ident[:BL, :BL], start=True, stop=True)
            capsT = rp.tile([128, 2 * BL], F32, tag="capsT")
            nc.vector.tensor_copy(capsT[:, 0:BL], pt1[:])
            nc.vector.tensor_copy(capsT[:32, BL:2 * BL], pt2[:])
            pf = ps_ef.tile([2, BL], F32, tag="pf", bufs=1)
            nc.tensor.matmul(pf[:], lhsT=wlin[:, 0, :], rhs=capsT[:, 0:BL],
                             start=True, stop=False)
            nc.tensor.matmul(pf[:], lhsT=wlin[:32, 1, :],
                             rhs=capsT[:32, BL:2 * BL],
                             start=False, stop=True)
            outT = rp.tile([2, BL], F32, tag="outT")
            nc.scalar.activation(outT[:], pf[:], AF.Identity,
                                 bias=blin[:, 0:1])
            dst = bass.AP(tensor=out_d, offset=0, ap=[[1, 2], [2, BL]])
            # issue from the Scalar queue: the Sync queue drains a long
            # event-semaphore backlog at kernel end (~9us) and would delay
            # this DMA; Scalar also produced outT, so this issues in-order
            nc.scalar.dma_start(out=dst, in_=outT[:])

    return nc


_CACHE = {}


def _get_nc(zero_bhn, zero_bx):
    key = (zero_bhn, zero_bx)
    if key not in _CACHE:
        nc = _build(zero_bhn, zero_bx)
        _split_waits(nc)   # HW-path legalization (CoreSim path builds its own)
        _CACHE[key] = nc
    return _CACHE[key]


def _host_inputs(x, emb, w_ih_f, w_hh_f, b_ih_f, b_hh_f,
                 w_ih_b, w_hh_b, b_ih_b, b_hh_b, W_cap, W_lin, b_lin):
    """Build the per-core input maps (everything but xidx is shared)."""
    f32 = np.float32
    bf16 = np.float16
    neg = np.ones((G3,), f32)
    neg[H:2 * H] = -1.0        # negate z gate (sigmoid -> 1-z)

    wih = np.stack([(w_ih_f.T * neg).astype(bf16), (w_ih_b.T * neg).astype(bf16)])
    whh = np.stack([(w_hh_f.T * neg).astype(bf16), (w_hh_b.T * neg).astype(bf16)])

    biasx = np.zeros((128, 6), f32)
    for d, (bi, bh) in enumerate([(b_ih_f, b_hh_f), (b_ih_b, b_hh_b)]):
        biasx[:, _BLKRZ[(d, 0)]] = (bi[0:H] + bh[0:H])
        biasx[:, _BLKRZ[(d, 1)]] = -(bi[H:2 * H] + bh[H:2 * H])
        biasx[:, 4 + d] = bi[2 * H:3 * H]
    bhn = np.zeros((128, 2), f32)
    bhn[:, 0] = b_hh_f[2 * H:3 * H]
    bhn[:, 1] = b_hh_b[2 * H:3 * H]
    zero_bhn = bool(np.all(bhn == 0.0))
    zero_bx = bool(np.all(biasx == 0.0))

    wcap = np.stack([W_cap[0:H, :].astype(bf16), W_cap[H:2 * H, :].astype(bf16)])
    selB = (np.arange(128)[:, None] % BL == np.arange(BL)[None, :]).astype(f32)
    selT = selB.T.copy()
    ident = np.eye(128, dtype=f32)

    shared = dict(emb=np.ascontiguousarray(emb).astype(bf16), wih=wih, whh=whh,
                  biasx=biasx, bhn=bhn, wcap=wcap,
                  wlin=np.ascontiguousarray(W_lin, f32),
                  blin=np.ascontiguousarray(b_lin, f32).reshape(2, 1),
                  selB=selB, selT=selT, ident=ident)

    in_maps = []
    for c in range(NCORES):
        xl = np.asarray(x[c * BL:(c + 1) * BL, :])          # [BL, S]
        tok = xl.T.reshape(-1).astype(np.int32)             # s-major [NTOK]
        xidx = np.ascontiguousarray(tok.reshape(NGRP, 128).T)  # [128, NGRP]
        in_maps.append(dict(shared, xidx=xidx))
    return in_maps, zero_bhn, zero_bx


def kernel(**inputs):
    in_maps, zero_bhn, zero_bx = _host_inputs(
        **{k: np.asarray(v) for k, v in inputs.items()})
    nc = _get_nc(zero_bhn, zero_bx)
    res = run_bass_kernel_spmd(nc, in_maps, list(range(NCORES)))
    return np.concatenate([res.results[c]["out"] for c in range(NCORES)],
                          axis=0)


def _install_ntff_hook():
    """Shim the missing antenv.axon_hooks so trace=True works under axon."""
    import sys, types
    if "antenv.axon_hooks" in sys.modules:
        return
    mod = types.ModuleType("antenv.axon_hooks")
    _h = [None]
    mod.set_axon_ntff_profile_hook = lambda h: _h.__setitem__(0, h)
    mod.get_axon_ntff_profile_hook = lambda: _h[0]
    sys.modules["antenv.axon_hooks"] = mod
    import antenv
    antenv.axon_hooks = mod
    from trn_agent_boot.trn_boot import _ntff_profile_via_ctypes
    mod.set_axon_ntff_profile_hook(
        _ntff_profile_via_ctypes("/opt/axon/libaxon_pjrt.so"))


def kernel_profiled(**inputs):
    """Same as kernel() but with NTFF tracing; returns (out, result_obj)."""
    _install_ntff_hook()
    in_maps, zero_bhn, zero_bx = _host_inputs(
        **{k: np.asarray(v) for k, v in inputs.items()})
    nc = _get_nc(zero_bhn, zero_bx)
    res = run_bass_kernel_spmd(nc, in_maps, list(range(NCORES)), trace=True)
    out = np.concatenate([res.results[c]["out"] for c in range(NCORES)],
                         axis=0)
    return out, res
